# revision 10
# baseline (speedup 1.0000x reference)
"""BiGRU Trainium2 kernel, 8-core SPMD, direction-split (4+4).

Cores 0-3 run the FWD GRU, cores 4-7 the BWD GRU (physical quads, same-die).
Each core owns a 256-wide h-slice of its direction, held as two 128-wide
subslices stacked on PSUM/SBUF partition halves (64 batch x 2 subs = 128
partitions) -- structurally identical tiles to the mixed baseline, so the
ACT/DVE gate pipeline is unchanged. Per step each core broadcasts its
transposed h tile [128 h x (64b sub0 | 64b sub1)] to its 3 quad peers + self
with FOUR relative SWDGE broadcast calls, one per SWDGE queue (one Q7 pair
each, descriptor generation pre-run LEAD steps ahead; only the trigger sits
in the step's critical path).

Phase-1 (xg = x @ Wih^T + bias) is uniform: one half-tile (384 gate cols of a
2-token tile) per step for all 512 steps; fwd cores consume x tiles in
forward order, bwd cores get a host-reversed copy of x, so the device
program is identical on all cores.

Final linear: each core computes out^T partials [128 o x 64 b] for its own
O-slice and its cross-pair's O-slice over its direction's h; the cross
partial is sent to the paired core on the other die (relative slot 6) and
added there.
"""

import os
import sys

sys.path.insert(0, "/opt/trn_rl_repo")

import numpy as np
import ml_dtypes

import concourse.bass as bass
import concourse.mybir as mybir

B = 64
T = 512
I = 1024
H = 1024
O = 1024
N = 8
NS = 4  # broadcast slots (quad size)
KT = 8  # 128-row contraction blocks in own-dir H (and I)
P = 128
SL = 128
G3 = 3 * SL  # gate cols per partition-half (r|z|n of one 128-subslice)

F_MAP = [0, 1, 2, 3, 6, 7, 4, 5]
FINV = [F_MAP.index(i) for i in range(8)]

BF16 = mybir.dt.bfloat16
F32 = mybir.dt.float32
AFT = mybir.ActivationFunctionType


def sigma_in(r: int, d: int) -> int:
    """In-quad index of the sender whose tile lands in slot d on core r."""
    return FINV[F_MAP[r] ^ d] % 4


def build_program(t_steps: int = T):
    from concourse.bacc import Bacc

    assert t_steps % 2 == 0
    NTT = t_steps // 2
    NH = t_steps  # ph1 half-tiles (one per step)
    LP = 6  # ph1 half-index lead over the step loop
    XPF = 4  # xg prefetch ring depth
    LEAD = 2  # broadcast descriptor pre-generation lead (steps)
    NQ = 4

    nc = Bacc(num_swdge_queues=NQ)

    xT = nc.declare_dram_parameter("xT", [NTT, P, KT * P], BF16, isOutput=False)
    wih = nc.declare_dram_parameter("wih", [KT, P, 2 * G3], BF16, isOutput=False)
    whh = nc.declare_dram_parameter("whh", [KT, P, 2 * G3], BF16, isOutput=False)
    wlin = nc.declare_dram_parameter("wlin", [2 * KT, P, SL], BF16, isOutput=False)
    bias1 = nc.declare_dram_parameter("bias1", [1, 2 * G3], BF16, isOutput=False)
    biasn = nc.declare_dram_parameter("biasn", [1, 2 * SL], BF16, isOutput=False)
    blin = nc.declare_dram_parameter("blin", [1, SL], BF16, isOutput=False)
    ident = nc.declare_dram_parameter("ident", [P, P], BF16, isOutput=False)
    ones = nc.declare_dram_parameter("ones", [1, P], BF16, isOutput=False)
    out = nc.declare_dram_parameter("out", [SL, B], F32, isOutput=True)

    XSR = 8  # xst SBUF ring depth in half-tiles (>= LP + 2)

    n_init_dma = KT + KT + 2 * KT + 5

    from contextlib import ExitStack

    es = ExitStack()
    with es:
        sem = lambda name: es.enter_context(nc.semaphore(name))
        sbuf = lambda name, shape, dt=BF16: es.enter_context(
            nc.sbuf_tensor(name, shape, dt)
        )
        psum = lambda name, shape, dt: es.enter_context(nc.psum_tensor(name, shape, dt))

        block = es.enter_context(nc.Block())
        init_sem = sem("init_sem")
        hz_sem = sem("hz_sem")
        bar_sem = sem("bar_sem")
        bar_p = sem("bar_p")
        bar_l = sem("bar_l")
        rsem = [[sem(f"rsem{par}_{d}") for d in range(NS)] for par in range(2)]
        lsem = [sem(f"lsem{q}") for q in range(NQ)]
        prep_q = [sem(f"prep_q{q}") for q in range(NQ)]
        psum_rdy = sem("psum_rdy")
        a2v_r = sem("a2v_r")
        a2v_z = sem("a2v_z")
        a2v_n = sem("a2v_n")
        v2a_np = sem("v2a_np")
        pf_v = sem("pf_v")
        v2p = sem("v2p")
        vch = sem("vch")
        p2v = sem("p2v")
        tdone = sem("tdone")
        xgc_p = sem("xgc_p")
        xn_p = sem("xn_p")
        a2v_xn = sem("a2v_xn")
        xt_dma = [sem("xt_dma0"), sem("xt_dma1")]
        p1_rdy = sem("p1_rdy")
        p1_cp = sem("p1_cp")
        lin_rdy = sem("lin_rdy")
        xch_s = sem("xch_s")
        rsem_x = sem("rsem_x")
        lsem_x = sem("lsem_x")
        prep_x = sem("prep_x")
        fin_sem = sem("fin_sem")

        whh_s = sbuf("whh_s", [P, KT * 2 * G3])
        wih_s = sbuf("wih_s", [P, KT * 2 * G3])
        wlin_s = sbuf("wlin_s", [P, 2 * KT * SL])
        hbuf = sbuf("hbuf", [P, 2 * NS * P])
        xn_s = sbuf("xn_s", [P, 2 * SL])
        xt_s = sbuf("xt_s", [P, 2 * KT * P])
        rz_s = sbuf("rz_s", [P, 2 * SL])
        t1_s = sbuf("t1_s", [P, SL])
        npre_s = sbuf("npre_s", [P, SL])
        n_s = sbuf("n_s", [P, SL])
        s1_s = sbuf("s1_s", [P, SL])
        s2_s = sbuf("s2_s", [P, SL])
        hst_s = sbuf("hst_s", [P, SL])
        hgn_s = sbuf("hgn_s", [P, SL])
        tb_s = sbuf("tb_s", [P, 2 * P])
        xst = sbuf("xst", [P, XSR * G3])
        ident_s = sbuf("ident_s", [P, P])
        ones_s = sbuf("ones_s", [1, P])
        bias1_s = sbuf("bias1_s", [1, 2 * G3])
        biasn_s = sbuf("biasn_s", [1, 2 * SL])
        blin_s = sbuf("blin_s", [1, SL])
        lino_s = sbuf("lino_s", [P, B], F32)
        linx_s = sbuf("linx_s", [P, B], F32)
        linr_s = sbuf("linr_s", [P, B], F32)
        out_s = sbuf("out_s", [P, B], F32)
        ps_rec0 = psum("ps_rec0", [P, G3], F32)
        ps_rec1 = psum("ps_rec1", [P, G3], F32)
        ps_tt = psum("ps_tt", [P, 2 * P], BF16)
        ps_p10 = psum("ps_p10", [P, G3], F32)
        ps_p11 = psum("ps_p11", [P, G3], F32)
        ps_xn = psum("ps_xn", [P, 2 * SL], F32)
        ps_lin = psum("ps_lin", [P, 2 * B], F32)
        ps_rec = [ps_rec0, ps_rec1]
        ps_t = [ps_tt[:, 0:P], ps_tt[:, P : 2 * P]]
        ps_p1 = [ps_p10, ps_p11]
        ps_lo = ps_lin[:, 0:B]
        ps_lx = ps_lin[:, B : 2 * B]

        def hb(t):
            return (t % 2) * NS * P

        # ---------------- SYNC: HWDGE DMA ---------------------------------
        @block.sync
        def _(s):
            for k in range(KT):
                s.dma_start(
                    out=whh_s[:, k * 2 * G3 : (k + 1) * 2 * G3], in_=whh[k, :, :]
                ).then_inc(init_sem, 16)
                s.dma_start(
                    out=wih_s[:, k * 2 * G3 : (k + 1) * 2 * G3], in_=wih[k, :, :]
                ).then_inc(init_sem, 16)
            for k in range(2 * KT):
                s.dma_start(
                    out=wlin_s[:, k * SL : (k + 1) * SL], in_=wlin[k, :, :]
                ).then_inc(init_sem, 16)
            s.dma_start(out=ident_s[:, :], in_=ident[:, :]).then_inc(init_sem, 16)
            s.dma_start(out=ones_s[:, :], in_=ones[:, :]).then_inc(init_sem, 16)
            s.dma_start(out=bias1_s[:, :], in_=bias1[:, :]).then_inc(init_sem, 16)
            s.dma_start(out=biasn_s[:, :], in_=biasn[:, :]).then_inc(init_sem, 16)
            s.dma_start(out=blin_s[:, :], in_=blin[:, :]).then_inc(init_sem, 16)

            def load_xt(p):
                if p >= NTT or p < 0:
                    return
                if load_xt.done >= p + 1:
                    return
                load_xt.done = p + 1
                if p >= 2:
                    s.wait_ge(p1_rdy, 2 * p - 2)  # tile p-2 fully consumed
                s.dma_start(
                    out=xt_s[:, (p % 2) * KT * P : ((p % 2) + 1) * KT * P],
                    in_=xT[p, :, :],
                ).then_inc(xt_dma[p % 2], 16)

            load_xt.done = 0

            # xg never leaves SBUF: ph1 output is consumed from the xst ring
            # by PE identity-matmul injects. Sync only streams the x tiles.
            load_xt(0)
            load_xt(1)
            load_xt(2)
            load_xt(3)
            for t in range(t_steps):
                if (t + LP) % 2 == 0:
                    load_xt((t + LP) // 2 + 1)

            s.wait_ge(fin_sem, 1)
            s.dma_start(out=out[:, :], in_=out_s[:, :]).then_inc(fin_sem, 16)

        # ---------------- PE ------------------------------------------------
        @block.tensor
        def _(pe):
            def ph1_half(h):
                if h >= NH:
                    return
                p, c = h // 2, h % 2
                pe.wait_ge(xt_dma[p % 2], 16 * (p // 2 + 1))
                if h >= 2:
                    pe.wait_ge(p1_cp, h - 1)  # psum h-2 copied out
                ps = ps_p1[h % 2]
                xo = (p % 2) * KT * P
                for k in range(KT):
                    pe.matmul(
                        ps[:, :],
                        xt_s[:, xo + k * P : xo + (k + 1) * P],
                        wih_s[:, k * 2 * G3 + c * G3 : k * 2 * G3 + (c + 1) * G3],
                        start=(k == 0),
                        stop=False,
                    )
                pe.matmul(
                    ps[:, :],
                    ones_s[0:1, :],
                    bias1_s[0:1, c * G3 : (c + 1) * G3],
                    start=False,
                    stop=True,
                ).then_inc(p1_rdy, 1)

            pe.wait_ge(init_sem, 16 * n_init_dma)
            pe.wait_ge(hz_sem, 2)
            for h in range(LP):
                ph1_half(h)

            for t in range(t_steps):
                ps = ps_rec[t % 2]
                if t >= 2:
                    pe.wait_ge(a2v_z, 2 * (t - 1))
                    pe.wait_ge(pf_v, t - 1)
                # stage xg_n for this step's token into ps_xn (ACT copies it
                # to SBUF for the DVE npre add) -- early, off the gate chain
                he = 2 * (t // 2)
                pr = (t % 2) * B
                se = (he % XSR) * G3
                so = ((he + 1) % XSR) * G3
                idb = ident_s[pr : pr + B, pr : pr + B]
                xno = (t % 2) * SL
                pe.wait_ge(p1_cp, he + 2)
                if t >= 2:
                    pe.wait_ge(a2v_xn, t - 1)
                pe.matmul(
                    ps_xn[0:B, xno : xno + SL],
                    idb,
                    xst[pr : pr + B, se + 2 * SL : se + G3],
                    start=True,
                    stop=True,
                )
                pe.matmul(
                    ps_xn[B:P, xno : xno + SL],
                    idb,
                    xst[pr : pr + B, so + 2 * SL : so + G3],
                    start=True,
                    stop=True,
                    skip_group_check=True,
                ).then_inc(xn_p, 1)
                hbo = hb(t)
                for kb in range(2 * NS):
                    d = kb // 2
                    if t >= 1 and kb % 2 == 0:
                        pe.wait_ge(rsem[(t - 1) % 2][d], 2 * ((t - 1) // 2 + 1))
                    lt = hbuf[:, hbo + kb * B : hbo + (kb + 1) * B]
                    pe.matmul(
                        ps[0:B, :],
                        lt,
                        whh_s[:, kb * 2 * G3 : kb * 2 * G3 + G3],
                        start=(kb == 0),
                        stop=(kb == 2 * NS - 1),
                    )
                    pe.matmul(
                        ps[B:P, :],
                        lt,
                        whh_s[:, kb * 2 * G3 + G3 : (kb + 1) * 2 * G3],
                        start=(kb == 0),
                        stop=(kb == 2 * NS - 1),
                        skip_group_check=True,
                    )
                pe.matmul(
                    ps[0:B, 0 : 2 * SL],
                    idb,
                    xst[pr : pr + B, se : se + 2 * SL],
                    start=False,
                    stop=False,
                    skip_group_check=True,
                )
                pe.matmul(
                    ps[B:P, 0 : 2 * SL],
                    idb,
                    xst[pr : pr + B, so : so + 2 * SL],
                    start=False,
                    stop=False,
                    skip_group_check=True,
                ).then_inc(xgc_p, 1)
                pe.matmul(
                    ps[0:B, 2 * SL : G3],
                    ones_s[0:1, 0:B],
                    biasn_s[0:1, 0:SL],
                    start=False,
                    stop=False,
                    skip_group_check=True,
                )
                pe.matmul(
                    ps[B:P, 2 * SL : G3],
                    ones_s[0:1, B:P],
                    biasn_s[0:1, SL : 2 * SL],
                    start=False,
                    stop=False,
                    skip_group_check=True,
                ).then_inc(psum_rdy, 1)

                ph1_half(t + LP)

                pe.wait_ge(v2p, t + 1)
                pe.transpose(ps_t[t % 2], hst_s[:, :], ident_s[:, :]).then_inc(
                    p2v, 1
                )

            # final linear partials over own-direction h
            for d in range(NS):
                pe.wait_ge(rsem[(t_steps - 1) % 2][d], 2 * ((t_steps - 1) // 2 + 1))
            hbo = hb(t_steps)
            for kb in range(2 * NS):
                pe.matmul(
                    ps_lo,
                    wlin_s[:, kb * SL : (kb + 1) * SL],
                    hbuf[:, hbo + kb * B : hbo + (kb + 1) * B],
                    start=(kb == 0),
                    stop=False,
                )
            pe.matmul(
                ps_lo,
                blin_s[0:1, :],
                ones_s[0:1, 0:B],
                start=False,
                stop=True,
            ).then_inc(lin_rdy, 1)
            for kb in range(2 * NS):
                mm = pe.matmul(
                    ps_lx,
                    wlin_s[:, (KT + kb) * SL : (KT + kb + 1) * SL],
                    hbuf[:, hbo + kb * B : hbo + (kb + 1) * B],
                    start=(kb == 0),
                    stop=(kb == 2 * NS - 1),
                )
            mm.then_inc(lin_rdy, 1)

        # ---------------- ACT ----------------------------------------------
        @block.scalar
        def _(a):
            for t in range(t_steps):
                ps = ps_rec[t % 2]
                xno = (t % 2) * SL
                a.wait_ge(xn_p, t + 1)
                a.activation(
                    xn_s[:, xno : xno + SL], ps_xn[:, xno : xno + SL], AFT.Copy
                ).then_inc(a2v_xn, 1)
                a.wait_ge(psum_rdy, t + 1)
                a.activation(rz_s[:, 0:SL], ps[:, 0:SL], AFT.Sigmoid).then_inc(
                    a2v_r, 1
                )
                a.activation(
                    rz_s[:, SL : 2 * SL], ps[:, SL : 2 * SL], AFT.Sigmoid
                ).then_inc(a2v_z, 1)
                a.activation(hgn_s[:, :], ps[:, 2 * SL : G3], AFT.Copy).then_inc(
                    a2v_z, 1
                )
                a.wait_ge(v2a_np, t + 1)
                a.activation(n_s[:, :], npre_s[:, :], AFT.Tanh).then_inc(a2v_n, 1)

        # ---------------- DVE ----------------------------------------------
        @block.vector
        def _(v):
            v.memset(hbuf[:, :], 0.0).then_inc(hz_sem, 1)
            v.memset(hst_s[:, :], 0.0).then_inc(hz_sem, 1)
            v.wait_ge(hz_sem, 2)

            def ph1_copy(h):
                if h >= NH:
                    return
                v.wait_ge(p1_rdy, h + 1)
                if h >= XSR:
                    # xst slot reuse: PE injects of the steps consuming half
                    # h-XSR (last one: 2*((h-XSR)//2)+1) must have completed
                    v.wait_ge(xgc_p, 2 * ((h - XSR) // 2) + 2)
                v.tensor_copy(
                    xst[:, (h % XSR) * G3 : (h % XSR + 1) * G3], ps_p1[h % 2][:, :]
                ).then_inc(p1_cp, 1)

            for h in range(LP):
                ph1_copy(h)

            for t in range(t_steps):
                ps = ps_rec[t % 2]
                xno = (t % 2) * SL
                v.wait_ge(a2v_r, t + 1)
                v.wait_ge(a2v_z, 2 * t + 2)
                v.tensor_mul(t1_s[:, :], rz_s[:, 0:SL], hgn_s[:, :]).then_inc(
                    pf_v, 1
                )
                v.wait_ge(pf_v, t + 1)
                v.wait_ge(a2v_xn, t + 1)
                v.tensor_add(
                    npre_s[:, :], t1_s[:, :], xn_s[:, xno : xno + SL]
                ).then_inc(v2a_np, 1)
                v.wait_ge(a2v_n, t + 1)
                if t >= 1:
                    v.wait_ge(v2p, t)
                v.tensor_sub(s1_s[:, :], hst_s[:, :], n_s[:, :]).then_inc(vch, 1)
                v.wait_ge(a2v_z, 2 * t + 1)
                v.wait_ge(vch, 2 * t + 1)
                v.tensor_mul(s2_s[:, :], rz_s[:, SL : 2 * SL], s1_s[:, :]).then_inc(
                    vch, 1
                )
                v.wait_ge(vch, 2 * t + 2)
                v.tensor_add(hst_s[:, :], n_s[:, :], s2_s[:, :]).then_inc(v2p, 1)

                v.wait_ge(p2v, t + 1)
                if t >= 2:
                    for q in range(NQ):
                        v.wait_ge(lsem[q], 16 * (t - 1))  # step t-2 sends done
                v.tensor_copy(
                    tb_s[:, (t % 2) * P : (t % 2) * P + P], ps_t[t % 2]
                ).then_inc(tdone, 1)

                ph1_copy(t + LP)

            # epilogue: stage cross partial, add received partner partial
            v.wait_ge(lin_rdy, 2)
            v.tensor_copy(linx_s[:, :], ps_lx).then_inc(xch_s, 1)
            v.tensor_copy(lino_s[:, :], ps_lo)
            v.wait_ge(rsem_x, 2)
            v.tensor_add(out_s[:, :], lino_s[:, :], linr_s[:, :]).then_inc(
                fin_sem, 1
            )

        # ---------------- GPSIMD: remote broadcasts -------------------------
        @block.gpsimd
        def _(g):
            g.wait_ge(hz_sem, 1)
            g.remote_sem_update_broadcast(
                remote_sem=bar_sem,
                local_sem=bar_l,
                rdests=[(0, k) for k in range(N)],
            ).then_inc(bar_p, 1)
            g.wait_ge(bar_p, 1)
            g.trigger_dma(count=1)
            g.wait_ge(bar_sem, 16)

            def descgen(s):
                if s >= t_steps:
                    return
                for d in range(NS):
                    rd = [None] * N
                    rd[d] = (0, d)
                    g.remote_dma_broadcast(
                        out_ap=hbuf[:, hb(s + 1) + d * P : hb(s + 1) + (d + 1) * P],
                        in_ap=tb_s[:, (s % 2) * P : (s % 2) * P + P],
                        remote_sem=rsem[s % 2][d],
                        local_sem=lsem[d],
                        rdests=rd,
                        queue_num=d,
                    ).then_inc(prep_q[d], 1)

            for s in range(LEAD):
                descgen(s)
            for t in range(t_steps):
                descgen(t + LEAD)
                g.wait_ge(psum_rdy, t + 1)
                g.wait_ge(tdone, t + 1)
                for q in range(NQ):
                    g.wait_ge(prep_q[q], t + 1)
                    g.trigger_dma(count=1, queue_num=q)

            # epilogue: cross-die partial exchange (relative slot 6)
            rd = [None] * N
            rd[6] = (0, 6)
            g.remote_dma_broadcast(
                out_ap=linr_s[:, :],
                in_ap=linx_s[:, :],
                remote_sem=rsem_x,
                local_sem=lsem_x,
                rdests=rd,
                queue_num=0,
            ).then_inc(prep_x, 1)
            g.wait_ge(xch_s, 1)
            g.wait_ge(prep_x, 1)
            g.trigger_dma(count=1, queue_num=0)

    nc.finalize()
    return nc


# ---- host-side input preparation ---------------------------------------------

BF16_NP = ml_dtypes.bfloat16


def _rows_g(i: int, s: int) -> np.ndarray:
    """Gate rows (r|z|n) of subslice s of in-quad core i, within 3H of one dir."""
    base = np.arange(128 * (2 * i + s), 128 * (2 * i + s) + 128)
    return np.concatenate([base, H + base, 2 * H + base])


def make_core_inputs(r, xT_f, xT_b, Wih_f, Whh_f, bih_f, bhh_f, Wih_b, Whh_b,
                     bih_b, bhh_b, W_lin, b_lin):
    g = r // 4
    i = r % 4
    if g == 0:
        Wih, Whh, bih, bhh, xTs = Wih_f, Whh_f, bih_f, bhh_f, xT_f
    else:
        Wih, Whh, bih, bhh, xTs = Wih_b, Whh_b, bih_b, bhh_b, xT_b
    partner = r + 4 if g == 0 else r - 4

    r0, r1 = _rows_g(i, 0), _rows_g(i, 1)

    def wih_pack():
        o = np.empty((KT, P, 2 * G3), dtype=BF16_NP)
        w0 = np.ascontiguousarray(Wih[r0, :].T)  # [I, 384]
        w1 = np.ascontiguousarray(Wih[r1, :].T)
        for k in range(KT):
            o[k, :, 0:G3] = w0[k * P : (k + 1) * P, :]
            o[k, :, G3 : 2 * G3] = w1[k * P : (k + 1) * P, :]
        return o

    def whh_pack():
        o = np.empty((KT, P, 2 * G3), dtype=BF16_NP)
        w0 = np.ascontiguousarray(Whh[r0, :].T)  # [H, 384]
        w1 = np.ascontiguousarray(Whh[r1, :].T)
        for kb in range(KT):
            d, su = kb // 2, kb % 2
            sj = sigma_in(r, d)
            hsl = slice(128 * (2 * sj + su), 128 * (2 * sj + su) + 128)
            o[kb, :, 0:G3] = w0[hsl, :]
            o[kb, :, G3 : 2 * G3] = w1[hsl, :]
        return o

    def wlin_pack():
        o = np.empty((2 * KT, P, SL), dtype=BF16_NP)
        for sl_i, orow in enumerate((r, partner)):
            wl = np.ascontiguousarray(
                W_lin[orow * SL : (orow + 1) * SL, g * H : (g + 1) * H].T
            )  # [H(own dir), 128]
            for kb in range(KT):
                d, su = kb // 2, kb % 2
                sj = sigma_in(r, d)
                hsl = slice(128 * (2 * sj + su), 128 * (2 * sj + su) + 128)
                o[sl_i * KT + kb] = wl[hsl, :]
        return o

    brz = bih + bhh
    b1 = np.empty((1, 2 * G3), dtype=BF16_NP)
    for c, rows in enumerate((r0, r1)):
        b1[0, c * G3 : c * G3 + 2 * SL] = brz[rows][0 : 2 * SL]
        b1[0, c * G3 + 2 * SL : (c + 1) * G3] = bih[rows][2 * SL : G3]

    bn = np.empty((1, 2 * SL), dtype=BF16_NP)
    bn[0, 0:SL] = bhh[r0][2 * SL : G3]
    bn[0, SL : 2 * SL] = bhh[r1][2 * SL : G3]

    return {
        "xT": xTs,
        "wih": wih_pack(),
        "whh": whh_pack(),
        "wlin": wlin_pack(),
        "bias1": b1,
        "biasn": bn,
        "blin": b_lin[r * SL : (r + 1) * SL].reshape(1, SL).astype(BF16_NP),
        "ident": np.eye(P, dtype=BF16_NP),
        "ones": np.ones((1, P), dtype=BF16_NP),
    }


def make_xT(input_btI: np.ndarray, t_steps: int = T) -> np.ndarray:
    """[B,T,I] -> [NTT, P, KT*P] bf16; tile p partitions = (token 2p | 2p+1)."""
    ntt = t_steps // 2
    xt = np.transpose(input_btI, (1, 0, 2))  # [T, B, I]
    v = xt.reshape(ntt, 2, B, KT, P)
    v = np.transpose(v, (0, 4, 3, 1, 2))  # [tau, i, k, toff, b]
    return np.ascontiguousarray(v.reshape(ntt, P, KT * P)).astype(BF16_NP)


_PROG_CACHE: dict = {}


def get_program(t_steps: int = T):
    if t_steps not in _PROG_CACHE:
        _PROG_CACHE[t_steps] = build_program(t_steps)
    return _PROG_CACHE[t_steps]


def kernel(input, Wih_f, Whh_f, bih_f, bhh_f, Wih_b, Whh_b, bih_b, bhh_b,
           W_lin, b_lin, t_steps: int = T):
    from concourse.bass_utils import run_bass_kernel_spmd

    args = [
        np.asarray(a, dtype=np.float32)
        for a in (Wih_f, Whh_f, bih_f, bhh_f, Wih_b, Whh_b, bih_b, bhh_b,
                  W_lin, b_lin)
    ]
    x = np.asarray(input, dtype=np.float32)[:, :t_steps, :]
    xT_f = make_xT(x, t_steps)
    xT_b = make_xT(x[:, ::-1, :], t_steps)
    nc = get_program(t_steps)
    in_maps = [make_core_inputs(r, xT_f, xT_b, *args) for r in range(N)]
    rr = run_bass_kernel_spmd(nc, in_maps, list(range(N)), **globals().get("RUN_KW", {}))
    res = rr.results
    global LAST_EXEC_NS, LAST_TRACE
    LAST_EXEC_NS = rr.exec_time_ns
    LAST_TRACE = rr.instructions_and_trace
    out = np.concatenate([res[r]["out"].T for r in range(N)], axis=1)
    return np.ascontiguousarray(out).astype(np.float32)
